# revision 35
# baseline (speedup 1.0000x reference)
"""Distributed multi-head attention block for 8 TRN2 NeuronCores.

Head-parallel sharding (2 heads/core) with an fp8e4m3 DoubleRow compute
core: all matmuls (q/k/v projections, scores, att@v, out-projection) run in
DoubleRow mode (0.5 cycles/row, 2 contraction subtiles/pass), ~2.8x less PE
work than bf16. The attention value tiles carry a 1/64 ones-column so the
softmax denominator accumulates into row 64 of the att@v PSUM tile for
free; normalization multiplies by 64/den (partition_broadcast of the
reciprocal row), which doubles as the x64 scale keeping fp8 out-proj
operands out of the denormal range. Weights are host-scaled x16; LayerNorm
rescales the collective result by 2^-10.

Softmax exp (the largest single cost: 256 units of [128, 1024]) is split
across ACT (true exp -> fp8) and DVE (Schraudolph integer exp writing fp8
bit patterns: bits = score*1.4427 + 56.15 truncated to int8); GPSIMD
cannot read PSUM so Pool only runs the reciprocal broadcast and the fp8
ReduceScatter (2 chunks [3072, 1024]; the big chunk amortizes the 15us
collective constant, the small one keeps the tail short).

Pipeline structure: the score PSUM is a depth-3 ring shared by both heads
(scores unit n -> tag n%3), giving the exp engines ~1.5 kc of lookahead so
neither in-order engine queue head-of-line-blocks the other. That depth is
paid for by keeping only ONE att@v accumulator live in the loop: head 0
streams in-loop (deferred past the previous tile's out-proj, which rides
the same 4KB PSUM tag), while head 1 is replayed from the retained e-pair
tiles at the start of the next tile's loop (32 cheap DoubleRow matmuls).
LayerNorm uses scalar_tensor_tensor with accum_out for sum stats on DVE
and ACT's Square activation with accumulate for the variance term;
projection bias evicts split between DVE (tensor_scalar) and ACT
(Identity activation with AP bias).
"""

import os
import sys

for _p in ("/opt/trn_rl_repo", "/root/.axon_site/_ro/trn_rl_repo"):
    if os.path.isdir(_p) and _p not in sys.path:
        sys.path.insert(0, _p)

import numpy as np
import ml_dtypes

import concourse.bass as bass
import concourse.mybir as mybir
import concourse.tile as tile
from concourse import bacc
from concourse.bass_utils import run_bass_kernel_spmd

# Problem dims
NQ = NK = 4096
D = 1024
H = 16
DA = 64

N_CORES = 8
HD = 128              # hd dims per core (2 heads x 64)
QT = 1024             # q tile
NQT = NQ // QT        # 4
KC = 128              # k chunk (partition axis of scores psum)
NKC = NK // KC        # 32
NPAIR = NKC // 2      # 16 DoubleRow k-chunk pairs
DC = 128              # d_in chunk for projections
NDC = D // DC         # 8
ROWS = NQ // N_CORES  # 512 output rows per core

# ReduceScatter chunks (rows of the 4096 q space)
CHUNKS = [(0, 3072), (3072, 1024)]

F32 = mybir.dt.float32
BF16 = mybir.dt.bfloat16
I8 = mybir.dt.int8
FP8 = mybir.dt.float8e4
FP8NP = ml_dtypes.float8_e4m3
DRM = mybir.MatmulPerfMode.DoubleRow

W_SCALE = 16.0        # host scale on weight matrices (fp8 normal range)
AO_SCALE = 64.0       # carried by ao via the 1/64 ones-column denominator
CC_SCALE = W_SCALE * AO_SCALE  # cc partials = CC_SCALE * attn contribution
VSLOT = 96            # per-head column slot in v tiles (64 v + ones + pad)

# Schraudolph fp8 exp: bits = sc*SCH_A + SCH_B (truncated to int8)
SCH_A = 8.0 * 0.125 / float(np.log(2.0))
SCH_B = 56.15

AVD_SLOT = 19         # kc slot where the h0 att@v psum tile is allocated

_COMPILED = None


def _build(identity_affine=False,
           exp_pattern=("act", "dve", "act", "dve", "act", "dve", "act",
                        "dve", "act", "dve", "act", "dve", "act", "act",
                        "dve", "act"),
           po_pattern=("act", "dve"),
           epi_slots=(1, 4, 5, 6, 7, 9, 10, 11, 12, 13, 14, 15, 16, 17, 18,
                      20, 22, 24, 26)):
    nc = bacc.Bacc("TRN2", target_bir_lowering=False, debug=False,
                   num_devices=N_CORES)

    # fp8 transposed inputs, [128, NDC, seq] (d-chunk-major)
    x8 = nc.dram_tensor("x8", [DC, NDC, NQ], FP8, kind="ExternalInput").ap()
    k8 = nc.dram_tensor("k8", [DC, NDC, NK], FP8, kind="ExternalInput").ap()
    v8 = nc.dram_tensor("v8", [DC, NDC, NK], FP8, kind="ExternalInput").ap()
    wq8 = nc.dram_tensor("wq8", [DC, NDC, HD], FP8, kind="ExternalInput").ap()
    wk8 = nc.dram_tensor("wk8", [DC, NDC, HD], FP8, kind="ExternalInput").ap()
    wv8 = nc.dram_tensor("wv8", [DC, NDC, HD], FP8, kind="ExternalInput").ap()
    wo8 = nc.dram_tensor("wo8", [DA, 2, D], FP8, kind="ExternalInput").ap()
    bq = nc.dram_tensor("bq", [HD, 1], F32, kind="ExternalInput").ap()
    bk = nc.dram_tensor("bk", [HD, 1], F32, kind="ExternalInput").ap()
    bv = nc.dram_tensor("bv", [HD, 1], F32, kind="ExternalInput").ap()
    resid = nc.dram_tensor("resid", [ROWS, D], F32, kind="ExternalInput").ap()
    gamma_b = nc.dram_tensor("gamma_b", [128, D], F32, kind="ExternalInput").ap()
    beta_b = nc.dram_tensor("beta_b", [128, D], F32, kind="ExternalInput").ap()
    out = nc.dram_tensor("out", [ROWS, D], F32, kind="ExternalOutput").ap()

    def eng(name):
        return {"act": nc.scalar, "dve": nc.vector, "pool": nc.gpsimd}[name]

    with tile.TileContext(nc) as tc:
      with tc.tile_pool(name="persist", bufs=1) as pp:
        # scores-DR layout: [32 (d%32), 2 (d-half), 2 (head), QT]
        qT_t = [pp.tile([32, 2, 2, QT], FP8, name=f"qT{i}") for i in range(NQT)]
        kT_t = [pp.tile([32, 2, 2, QT], FP8, name=f"kT{i}") for i in range(NQT)]
        # v tile i holds k-chunks 8i..8i+7; per chunk per head: 64 v cols at
        # h*VSLOT, a 1/64 ones col at h*VSLOT+64 (denominator), pad to VSLOT.
        v_t = [pp.tile([128, 8, 2 * VSLOT], FP8, name=f"v{i}")
               for i in range(NQT)]
        ao_t = [pp.tile([DA, 2, QT], FP8, name=f"ao{i}", bufs=2, tag="ao")
                for i in range(NQT)]
        wq_sb = pp.tile([DC, NDC, HD], FP8, name="wq_sb")
        wk_sb = pp.tile([DC, NDC, HD], FP8, name="wk_sb")
        wv_sb = pp.tile([DC, NDC, HD], FP8, name="wv_sb")
        wo_sb = pp.tile([DA, 2, D], FP8, name="wo_sb")
        bq_sb = pp.tile([HD, 1], F32, name="bq_sb")
        bk_sb = pp.tile([HD, 1], F32, name="bk_sb")
        bv_sb = pp.tile([HD, 1], F32, name="bv_sb")
        gam_sb = pp.tile([128, D], F32, name="gam_sb")
        bet_sb = pp.tile([128, D], F32, name="bet_sb")

        nc.sync.dma_start(wk_sb[:], wk8)
        nc.sync.dma_start(wq_sb[:], wq8)
        nc.sync.dma_start(wv_sb[:], wv8)
        nc.sync.dma_start(bq_sb[:], bq)
        nc.sync.dma_start(bk_sb[:], bk)
        nc.sync.dma_start(bv_sb[:], bv)
        for i in range(NQT):
            for h in range(2):
                nc.vector.memset(
                    v_t[i][:, :, h * VSLOT + DA:h * VSLOT + DA + 1],
                    1.0 / AO_SCALE)

        with tc.tile_pool(name="io", bufs=3) as io, \
             tc.tile_pool(name="vio", bufs=2) as vio, \
             tc.tile_pool(name="et", bufs=20) as et, \
             tc.tile_pool(name="misc", bufs=4) as misc, \
             tc.tile_pool(name="ln", bufs=1) as lnp, \
             tc.tile_pool(name="ps", bufs=1, space="PSUM") as ps, \
             tc.tile_pool(name="dram", bufs=1, space="DRAM") as dram:

            cc_in = dram.tile([NQ, D], FP8, name="cc_in")
            cc_outs = [dram.tile([n // N_CORES, D], FP8, name=f"cc_out{i}")
                       for i, (_, n) in enumerate(CHUNKS)]

            # ---------- projection units ----------
            def fold_qk(dst_tile, src_tile, col0, ncols):
                # [128, ncols] fp8 -> [32, 2, 2, ncols] partition fold via DMA
                for g in range(4):
                    h, dh = g // 2, g % 2
                    nc.sync.dma_start(
                        dst_tile[:, dh, h, col0:col0 + ncols],
                        src_tile[g * 32:(g + 1) * 32, 0:ncols])

            def load_qk(src_dram, t, key):
                xt = io.tile([DC, NDC, QT], FP8, tag="xt", name=f"xt_{key}")
                nc.sync.dma_start(xt[:],
                                  src_dram[:, :, t * QT:(t + 1) * QT])
                return xt

            def proj_qk(dst_tile, w_sb, b_sb, xt, t, tag):
                psum = ps.tile([HD, QT], F32, tag=tag, name=f"pp_{tag}_{t}")
                for j in range(4):
                    for half in range(2):
                        nc.tensor.matmul(
                            psum[:, half * 512:(half + 1) * 512],
                            lhsT=w_sb[:, 2 * j:2 * j + 2, :],
                            rhs=xt[:, 2 * j:2 * j + 2,
                                   half * 512:(half + 1) * 512],
                            start=(j == 0), stop=(j == 3), perf_mode=DRM)
                tmp = misc.tile([HD, QT], FP8, tag="qktmp",
                                name=f"qkt_{tag}_{t}")
                nc.scalar.activation(
                    tmp[:], psum[:], mybir.ActivationFunctionType.Identity,
                    bias=b_sb[:], scale=1.0 / W_SCALE)
                fold_qk(dst_tile, tmp, 0, QT)

            def proj_k_mini():
                # k columns 0:KC only, to unblock the first score matmul
                psum = ps.tile([HD, KC], F32, tag="sc0", name="pk_mini")
                xt = io.tile([DC, NDC, KC], FP8, tag="xtm", name="xtm")
                nc.sync.dma_start(xt[:], k8[:, :, 0:KC])
                for j in range(4):
                    nc.tensor.matmul(
                        psum[:], lhsT=wk_sb[:, 2 * j:2 * j + 2, :],
                        rhs=xt[:, 2 * j:2 * j + 2, :],
                        start=(j == 0), stop=(j == 3), perf_mode=DRM)
                tmp = misc.tile([HD, KC], FP8, tag="qktmp", name="pkm_t")
                nc.scalar.activation(
                    tmp[:], psum[:], mybir.ActivationFunctionType.Identity,
                    bias=bk_sb[:], scale=1.0 / W_SCALE)
                fold_qk(kT_t[0], tmp, 0, KC)

            def load_k0_rest():
                xt = io.tile([DC, NDC, QT - KC], FP8, tag="xt", name="xtr")
                nc.sync.dma_start(xt[:], k8[:, :, KC:QT])
                return xt

            def proj_k0_rest(xt):
                psum = ps.tile([HD, QT - KC], F32, tag="sc1", name="pk_rest")
                for j in range(4):
                    for c0, w in ((0, 448), (448, 448)):
                        nc.tensor.matmul(
                            psum[:, c0:c0 + w],
                            lhsT=wk_sb[:, 2 * j:2 * j + 2, :],
                            rhs=xt[:, 2 * j:2 * j + 2, c0:c0 + w],
                            start=(j == 0), stop=(j == 3), perf_mode=DRM)
                tmp = misc.tile([HD, QT - KC], FP8, tag="qktmp", name="pkr_t")
                nc.scalar.activation(
                    tmp[:], psum[:], mybir.ActivationFunctionType.Identity,
                    bias=bk_sb[:], scale=1.0 / W_SCALE)
                fold_qk(kT_t[0], tmp, KC, QT - KC)

            def load_v(u):
                xv = vio.tile([DC, NDC, 512], FP8, tag="xv", name=f"xv_{u}")
                nc.sync.dma_start(xv[:], v8[:, :, u * 512:(u + 1) * 512])
                return xv

            def proj_v(u, xv, tag):
                # chunks 4u..4u+3: 16 DR matmuls into a [128, 512] psum,
                # two ACT Identity bias evicts.
                psum = ps.tile([128, 512], F32, tag=tag, name=f"vp_{u}")
                for c in range(4):
                    for j in range(4):
                        nc.tensor.matmul(
                            psum[:, c * KC:(c + 1) * KC],
                            lhsT=xv[:, 2 * j:2 * j + 2, c * KC:(c + 1) * KC],
                            rhs=wv_sb[:, 2 * j:2 * j + 2, :],
                            start=(j == 0), stop=(j == 3), perf_mode=DRM)
                ti, lc0 = u // 2, (u % 2) * 4
                pv = psum[:].rearrange("p (c hd) -> p c hd", c=4)
                for h in range(2):
                    nc.scalar.activation(
                        v_t[ti][:, lc0:lc0 + 4, h * VSLOT:h * VSLOT + DA],
                        pv[:, :, h * DA:(h + 1) * DA],
                        mybir.ActivationFunctionType.Identity,
                        bias=bv_sb[:], scale=1.0 / W_SCALE)

            # ---------- epilogue units ----------
            def norm(qt, h, avd_h):
                rec = lnp.tile([1, QT], F32, tag=f"rec{h}",
                               name=f"rec{h}_{qt}")
                # DVE allows the 32-aligned partition shift 64 -> 0;
                # partition_broadcast only reads physical partition 0.
                nc.vector.reciprocal(rec[0:1, :], avd_h[DA:DA + 1, :])
                rb = lnp.tile([DA, QT], F32, tag=f"rb{h}", name=f"rb{h}_{qt}")
                nc.gpsimd.partition_broadcast(rb[:], rec[0:1, :], channels=DA)
                nc.vector.tensor_mul(
                    out=ao_t[qt][:, h, :], in0=avd_h[0:DA, :], in1=rb[:])

            def attv_pair(avd_h, h, pr, e_pair):
                ti, lc = pr // 4, 2 * (pr % 4)
                for half in range(2):
                    nc.tensor.matmul(
                        avd_h[:, half * 512:(half + 1) * 512],
                        lhsT=v_t[ti][:, lc:lc + 2,
                                     h * VSLOT:h * VSLOT + DA + 1],
                        rhs=e_pair[h][:, :, half * 512:(half + 1) * 512],
                        start=(pr == 0), stop=(pr == NPAIR - 1),
                        perf_mode=DRM, tile_position=(0, 0))

            def replay_h1(qt, pairs, st, p0, p1, tag="avd"):
                if p0 == 0:
                    st["avd1"] = ps.tile([DA + 1, QT], F32, tag=tag,
                                         name=f"avd1_{qt}")
                for pr in range(p0, p1):
                    attv_pair(st["avd1"], 1, pr, pairs[pr])

            def oproj_mm(qt, c, tag="avd"):
                op = ps.tile([128, D], F32, tag=tag, name=f"op_{qt}_{c}")
                for half in range(2):
                    nc.tensor.matmul(
                        op[:, half * 512:(half + 1) * 512],
                        lhsT=ao_t[qt][:, :, c * KC:(c + 1) * KC],
                        rhs=wo_sb[:, :, half * 512:(half + 1) * 512],
                        start=True, stop=True, perf_mode=DRM)
                return op

            def oproj_evict(qt, c, op):
                po = misc.tile([128, D], FP8, tag="po", name=f"po_{qt}_{c}")
                e = po_pattern[(qt * 8 + c) % len(po_pattern)]
                if e == "act":
                    nc.scalar.copy(out=po[:], in_=op[:])
                else:
                    eng(e).tensor_copy(out=po[:], in_=op[:])
                nc.sync.dma_start(
                    cc_in[qt * QT + c * KC:qt * QT + (c + 1) * KC, :], po[:])

            def oproj_step(qt, c, st, tag="avd"):
                # evict previous chunk's psum (long since computed, so the
                # copy never head-of-line-blocks an exp queue), then matmul
                # the next chunk into the freed tag slot.
                if c > 0:
                    oproj_evict(qt, c - 1, st.pop("op"))
                if c < 8:
                    st["op"] = oproj_mm(qt, c, tag)

            def rsqrt_newton(dst, var, tag, rows):
                y = lnp.tile([128, 1], F32, tag=f"ny{tag}", name=f"ny_{tag}")
                nc.vector.memset(y[:rows], 0.85)
                t = lnp.tile([128, 1], F32, tag=f"nt{tag}", name=f"nt_{tag}")
                for _ in range(3):
                    nc.vector.tensor_mul(out=t[:rows], in0=y[:rows],
                                         in1=y[:rows])
                    nc.vector.tensor_mul(out=t[:rows], in0=t[:rows], in1=var)
                    nc.vector.tensor_scalar(
                        out=t[:rows], in0=t[:rows], scalar1=-0.5, scalar2=1.5,
                        op0=mybir.AluOpType.mult, op1=mybir.AluOpType.add)
                    nc.vector.tensor_mul(out=y[:rows], in0=y[:rows],
                                         in1=t[:rows])
                nc.vector.tensor_copy(out=dst, in_=y[:rows])

            def ln_block(ci, b, rows):
                # one <=128-row striped LayerNorm block of chunk ci
                ost = sum(CHUNKS[j][1] // N_CORES for j in range(ci)) + b * 128
                tg = f"{ci}_{b}"
                rs = lnp.tile([128, D], FP8, tag="rs", name=f"rs_{tg}")
                nc.sync.dma_start(rs[:rows],
                                  cc_outs[ci][b * 128:b * 128 + rows, :])
                rd = lnp.tile([128, D], F32, tag="rd", name=f"rd_{tg}")
                nc.sync.dma_start(rd[:rows], resid[ost:ost + rows, :])
                y = lnp.tile([128, D], F32, tag="y", name=f"y_{tg}")
                mu1 = lnp.tile([128, 1], F32, tag="mu1", name=f"mu1_{tg}")
                nc.vector.scalar_tensor_tensor(
                    out=y[:rows], in0=rs[:rows], scalar=1.0 / CC_SCALE,
                    in1=rd[:rows], op0=mybir.AluOpType.mult,
                    op1=mybir.AluOpType.add, accum_out=mu1[:rows])
                s21 = lnp.tile([128, 1], F32, tag="s21", name=f"s21_{tg}")
                nc.scalar.activation(
                    rd[:rows], y[:rows], mybir.ActivationFunctionType.Square,
                    accum_out=s21[:rows])
                mu = lnp.tile([128, 1], F32, tag="mu", name=f"mu_{tg}")
                nc.vector.tensor_scalar_mul(mu[:rows], mu1[:rows], 1.0 / D)
                mu2 = lnp.tile([128, 1], F32, tag="mu2", name=f"mu2_{tg}")
                nc.vector.tensor_mul(out=mu2[:rows], in0=mu[:rows],
                                     in1=mu[:rows])
                var = lnp.tile([128, 1], F32, tag="var", name=f"var_{tg}")
                nc.vector.tensor_scalar(
                    out=var[:rows], in0=s21[:rows], scalar1=1.0 / D,
                    scalar2=mu2[:rows], op0=mybir.AluOpType.mult,
                    op1=mybir.AluOpType.subtract)
                rstd = lnp.tile([128, 1], F32, tag="rstd", name=f"rstd_{tg}")
                rsqrt_newton(rstd[:rows], var[:rows], tg, rows)
                xc = lnp.tile([128, D], F32, tag="xc", name=f"xc_{tg}")
                nc.vector.tensor_scalar(
                    out=xc[:rows], in0=y[:rows], scalar1=mu[:rows],
                    scalar2=rstd[:rows],
                    op0=mybir.AluOpType.subtract, op1=mybir.AluOpType.mult)
                if not identity_affine:
                    nc.vector.tensor_mul(out=xc[:rows], in0=xc[:rows],
                                         in1=gam_sb[:rows])
                    nc.vector.tensor_add(out=xc[:rows], in0=xc[:rows],
                                         in1=bet_sb[:rows])
                nc.sync.dma_start(out[ost:ost + rows, :], xc[:rows])

            def do_rs(ci):
                s, n = CHUNKS[ci]
                nc.gpsimd.collective_compute(
                    "ReduceScatter", mybir.AluOpType.add,
                    replica_groups=[list(range(N_CORES))],
                    ins=[cc_in[s:s + n, :].opt()],
                    outs=[cc_outs[ci][:].opt()])

            # ---------- schedule ----------
            proj_k_mini()
            proj_qk(qT_t[0], wq_sb, bq_sb, load_qk(x8, 0, "q0"), 0, "sc1")
            nc.sync.dma_start(wo_sb[:], wo8)
            nc.sync.dma_start(gam_sb[:], gamma_b)
            nc.sync.dma_start(bet_sb[:], beta_b)
            ld = {"k0r": load_k0_rest()}

            # Unit at slot s is emitted inside iteration s, BEFORE the attv
            # block but AFTER that iteration's scores. Every unit must
            # precede its first reader: kT_t[i] is read from kc=8i, v chunks
            # 2p..2p+1 by the h0 attv pass at kc=max(2p+2, AVD_SLOT+1), and
            # by the h1 replay early in the next tile's loop.
            QT0_SLOTS = {
                0: lambda: [proj_k0_rest(ld.pop("k0r"))],
                1: lambda: [ld.__setitem__("v0", load_v(0)),
                            ld.__setitem__("k1", load_qk(k8, 1, "k1"))],
                4: lambda: [proj_qk(kT_t[1], wk_sb, bk_sb, ld.pop("k1"),
                                    1, "sc0"),
                            ld.__setitem__("v1", load_v(1))],
                6: lambda: [proj_v(0, ld.pop("v0"), "sc1"),
                            ld.__setitem__("v2", load_v(2))],
                8: lambda: [proj_v(1, ld.pop("v1"), "sc2"),
                            ld.__setitem__("k2", load_qk(k8, 2, "k2"))],
                10: lambda: [proj_v(2, ld.pop("v2"), "sc0"),
                             ld.__setitem__("v3", load_v(3))],
                12: lambda: [proj_qk(kT_t[2], wk_sb, bk_sb, ld.pop("k2"),
                                     2, "sc1"),
                             ld.__setitem__("v4", load_v(4))],
                14: lambda: [proj_v(3, ld.pop("v3"), "sc2"),
                             ld.__setitem__("q1", load_qk(x8, 1, "q1"))],
                16: lambda: [proj_v(4, ld.pop("v4"), "sc0"),
                             ld.__setitem__("v5", load_v(5))],
                18: lambda: [proj_qk(qT_t[1], wq_sb, bq_sb, ld.pop("q1"),
                                     1, "sc1"),
                             ld.__setitem__("k3", load_qk(k8, 3, "k3"))],
                20: lambda: [proj_v(5, ld.pop("v5"), "sc2"),
                             ld.__setitem__("v6", load_v(6))],
                22: lambda: [proj_qk(kT_t[3], wk_sb, bk_sb, ld.pop("k3"),
                                     3, "sc0"),
                             ld.__setitem__("v7", load_v(7))],
                24: lambda: [proj_v(6, ld.pop("v6"), "sc1"),
                             ld.__setitem__("q2", load_qk(x8, 2, "q2"))],
                26: lambda: [proj_v(7, ld.pop("v7"), "sc2")],
                28: lambda: [proj_qk(qT_t[2], wq_sb, bq_sb, ld.pop("q2"),
                                     2, "sc0"),
                             ld.__setitem__("q3", load_qk(x8, 3, "q3"))],
                30: lambda: [proj_qk(qT_t[3], wq_sb, bq_sb, ld.pop("q3"),
                                     3, "sc1")],
            }

            epilogue = []
            exp_i = 0
            for qt in range(NQT):
                avd0 = None
                e_pairs = {}
                next_pair = 0
                for kc in range(NKC + 2):
                    if kc < NKC:
                        p = kc // 2
                        if kc % 2 == 0:
                            e_pairs[p] = [
                                et.tile([128, 2, QT], FP8, tag=f"e{h}",
                                        name=f"e{h}_{qt}_{p}")
                                for h in range(2)]
                        ktile, kcol = kc // 8, kc % 8
                        for h in range(2):
                            sc = ps.tile([KC, QT], F32,
                                         tag=f"sc{(2 * kc + h) % 3}",
                                         name=f"sc{h}_{qt}_{kc}")
                            for half in range(2):
                                nc.tensor.matmul(
                                    sc[:, half * 512:(half + 1) * 512],
                                    lhsT=kT_t[ktile][
                                        :, :, h, kcol * KC:(kcol + 1) * KC],
                                    rhs=qT_t[qt][
                                        :, :, h, half * 512:(half + 1) * 512],
                                    start=True, stop=True, perf_mode=DRM)
                            ename = exp_pattern[exp_i % len(exp_pattern)]
                            exp_i += 1
                            dst = e_pairs[p][h][:, kc % 2, :]
                            if ename == "act":
                                nc.scalar.activation(
                                    dst, sc[:],
                                    mybir.ActivationFunctionType.Exp,
                                    scale=0.125)
                            else:
                                nc.vector.tensor_scalar(
                                    out=dst.bitcast(I8), in0=sc[:],
                                    scalar1=SCH_A, scalar2=SCH_B,
                                    op0=mybir.AluOpType.mult,
                                    op1=mybir.AluOpType.add)
                    if epilogue and kc in epi_slots:
                        epilogue.pop(0)()
                    if qt == 0 and kc in QT0_SLOTS:
                        QT0_SLOTS[kc]()
                    if kc == AVD_SLOT:
                        avd0 = ps.tile([DA + 1, QT], F32, tag="avd",
                                       name=f"avd0_{qt}")
                    # h0 att@v on completed pairs (deferred past AVD_SLOT)
                    if avd0 is not None and kc % 2 == 0:
                        while next_pair <= kc // 2 - 1:
                            attv_pair(avd0, 0, next_pair, e_pairs[next_pair])
                            next_pair += 1
                st = {}
                epi = [lambda qt=qt, a=avd0: norm(qt, 0, a)]
                epi += [lambda qt=qt, ps_=e_pairs, st=st, g=g:
                        replay_h1(qt, ps_, st, 4 * g, 4 * g + 4)
                        for g in range(4)]
                epi += [lambda qt=qt, st=st: norm(qt, 1, st["avd1"])]
                epi += [lambda qt=qt, st=st, c=c: oproj_step(qt, c, st)
                        for c in range(9)]
                if qt == 2:
                    # fire RS1 early (overlaps qt3's loop); its LN blocks
                    # move to the tail where they overlap RS2 on Pool
                    epi += [lambda: do_rs(0)]
                epilogue = epi
            # tail: final tile's epilogue; h1 replay goes to a freed score
            # tag so it runs concurrently with norm h0, and out-proj
            # pipelines through the remaining free tags
            epilogue[1] = lambda: replay_h1(3, e_pairs, st, 0, 8, "sc0")
            epilogue[2] = lambda: replay_h1(3, e_pairs, st, 8, 16, "sc0")
            epilogue[3] = lambda: None
            epilogue[4] = lambda: None
            fin_tags = ("sc1", "sc2", "avd", "sc1", "sc2", "avd", "sc1",
                        "sc2")
            ops = {}
            def fin_oproj(c):
                if c > 0:
                    oproj_evict(3, c - 1, ops.pop(c - 1))
                if c < 8:
                    ops[c] = oproj_mm(3, c, fin_tags[c])
            for i in range(9):
                epilogue[6 + i] = lambda c=i: fin_oproj(c)
            for step in epilogue:
                step()
            do_rs(1)
            for b in range(3):
                ln_block(0, b, 128)
            ln_block(1, 0, 128)

    nc.compile()
    return nc


def _to8(a):
    return np.ascontiguousarray(a).astype(FP8NP)


def _shard(inputs):
    q = np.asarray(inputs["queries"], dtype=np.float32)
    k = np.asarray(inputs["keys"], dtype=np.float32)
    v = np.asarray(inputs["values"], dtype=np.float32)
    Wq = np.asarray(inputs["Wq"], dtype=np.float32)
    Wk = np.asarray(inputs["Wk"], dtype=np.float32)
    Wv = np.asarray(inputs["Wv"], dtype=np.float32)
    Wo = np.asarray(inputs["Wo"], dtype=np.float32)
    bq = np.asarray(inputs["bq"], dtype=np.float32)
    bk = np.asarray(inputs["bk"], dtype=np.float32)
    bv = np.asarray(inputs["bv"], dtype=np.float32)
    bo = np.asarray(inputs["bo"], dtype=np.float32)
    gamma = np.asarray(inputs["gamma"], dtype=np.float32)
    beta = np.asarray(inputs["beta"], dtype=np.float32)

    # [DC, NDC, seq]: element (p, j, n) = x[n, j*128+p]
    def tr8(a):
        return _to8(a.T.reshape(NDC, DC, a.shape[0]).transpose(1, 0, 2))

    x8 = tr8(q)
    k8_ = tr8(k)
    v8_ = tr8(v)
    gam_b = np.ascontiguousarray(
        np.broadcast_to(gamma, (128, D))).astype(np.float32)
    bet_b = np.ascontiguousarray(
        np.broadcast_to(beta, (128, D))).astype(np.float32)

    in_maps = []
    for c in range(N_CORES):
        hd = slice(c * HD, (c + 1) * HD)
        row_idx = np.concatenate(
            [np.arange(s + c * (n // N_CORES), s + (c + 1) * (n // N_CORES))
             for s, n in CHUNKS])
        in_maps.append({
            "x8": x8, "k8": k8_, "v8": v8_,
            "wq8": _to8((Wq[:, hd] * W_SCALE).reshape(NDC, DC, HD)
                        .transpose(1, 0, 2)),
            "wk8": _to8((Wk[:, hd] * W_SCALE).reshape(NDC, DC, HD)
                        .transpose(1, 0, 2)),
            "wv8": _to8((Wv[:, hd] * W_SCALE).reshape(NDC, DC, HD)
                        .transpose(1, 0, 2)),
            "wo8": _to8((Wo[hd, :] * W_SCALE).reshape(2, DA, D)
                        .transpose(1, 0, 2)),
            "bq": np.ascontiguousarray(bq[hd, None]),
            "bk": np.ascontiguousarray(bk[hd, None]),
            "bv": np.ascontiguousarray(bv[hd, None]),
            "resid": np.ascontiguousarray(q[row_idx, :] + bo[None, :]),
            "gamma_b": gam_b, "beta_b": bet_b,
        })
    return in_maps


def kernel(**inputs):
    global _COMPILED
    ident = bool(np.all(np.asarray(inputs["gamma"]) == 1.0)
                 and np.all(np.asarray(inputs["beta"]) == 0.0))
    if _COMPILED is None or _COMPILED[1] != ident:
        _COMPILED = (_build(identity_affine=ident), ident)
    nc = _COMPILED[0]
    in_maps = _shard(inputs)
    res = run_bass_kernel_spmd(nc, in_maps, core_ids=list(range(N_CORES)))
    full = np.empty((NQ, D), dtype=np.float32)
    for c in range(N_CORES):
        oc = res.results[c]["out"]
        ost = 0
        for s, n in CHUNKS:
            rch = n // N_CORES
            full[s + c * rch: s + (c + 1) * rch, :] = oc[ost:ost + rch, :]
            ost += rch
    return full
